# revision 9
# baseline (speedup 1.0000x reference)
"""CTRNN kernel for 8 Trainium2 NeuronCores (Bass/Tile).

Model (per reference):
    u = inputs @ W_in.T + b_in                      # (T,B,H)
    v_t = (1-a) v_{t-1} + a (u_t + fr_{t-1} @ W_h.T + b_h)
    fr_t = relu(v_t)                                # hidden_seq
    logits = hidden_seq @ W_out.T + b_out           # (T,B,O)

Sharding: data-parallel over batch B=128 -> 16 per core; the T=512
recurrence runs sequentially per core. All state is kept transposed
on-chip as [H on partitions (4 x 128 chunks), batch on free dim].
Matmul operands are bf16 (fp32 PSUM accumulation, fp32 state update);
fp32 matmuls on TRN2 lower to 2x half-rate passes and are ~11x slower
per the measured LDWEIGHTS+MATMUL cadence.
"""

import sys

sys.path.insert(0, "/opt/trn_rl_repo")

import ml_dtypes
import numpy as np

import concourse.bass as bass
import concourse.mybir as mybir
from concourse.bass_utils import run_bass_kernel_spmd
from concourse.tile import TileContext

T, B, I, H, O = 512, 128, 64, 512, 32
ALPHA = 10.0 / 100.0
NCORES = 8
BL = B // NCORES          # 16 batch per core
NH = H // 128             # 4 h chunks of 128 partitions
WIN = 32                  # timesteps per output window
NW = T // WIN             # 16 windows
NT = T * BL               # 8192 (t,b) rows per core
F32 = mybir.dt.float32
BF16 = mybir.dt.bfloat16
NPBF = ml_dtypes.bfloat16

# ---------------------------------------------------------------------------
# Workaround: this walrus build rejects instructions carrying more than one
# sem wait ("Too many sync wait commands"). After Tile scheduling, move
# excess waits onto same-engine NOPs inserted immediately before the
# offending instruction (same sequencer -> identical blocking semantics).
_MAX_WAITS = 1


def _split_excess_waits(nc: bass.Bass) -> None:
    count = 0
    for fn in nc.m.functions:
        for bb in fn.blocks:
            idx = 0
            while idx < len(bb.instructions):
                inst = bb.instructions[idx]
                si = inst.sync_info
                waits = list(si.on_wait) if si and si.on_wait else []
                if len(waits) > _MAX_WAITS:
                    keep = waits[-_MAX_WAITS:]
                    extra = waits[:-_MAX_WAITS]
                    inst.sync_info = mybir.SyncInfo(
                        on_wait=keep, on_update=list(si.on_update or [])
                    )
                    for w in extra:
                        count += 1
                        nop = mybir.InstNoOp(
                            name=f"I-waitsplit-{count}", ins=[], outs=[]
                        )
                        nop.engine = inst.engine
                        nop.sync_info = mybir.SyncInfo(on_wait=[w], on_update=[])
                        bb.instructions.insert(idx, nop)
                        idx += 1
                idx += 1
# ---------------------------------------------------------------------------

add = mybir.AluOpType.add
mult = mybir.AluOpType.mult


def _build_nc() -> bass.Bass:
    nc = bass.Bass()

    # Per-core inputs (already transposed/scaled/cast on host).
    xT = nc.declare_dram_parameter("xT", [I, NT], BF16, isOutput=False)
    whT = nc.declare_dram_parameter("whT", [NH, 128, H], BF16, isOutput=False)
    wiT = nc.declare_dram_parameter("wiT", [I, H], BF16, isOutput=False)
    woT = nc.declare_dram_parameter("woT", [NH, 128, O], BF16, isOutput=False)
    bh = nc.declare_dram_parameter("bh", [128, NH], F32, isOutput=False)
    bo = nc.declare_dram_parameter("bo", [O, 1], F32, isOutput=False)
    hseqT = nc.declare_dram_parameter(
        "hseqT", [NW, 128, NH * WIN * BL], BF16, isOutput=True
    )
    logT = nc.declare_dram_parameter(
        "logT", [NW, O, WIN * BL], F32, isOutput=True
    )

    with TileContext(nc) as tc:
        with (
            tc.tile_pool(name="const", bufs=1) as constp,
            tc.tile_pool(name="uproj", bufs=1) as uprojp,
            tc.tile_pool(name="xstage", bufs=3) as xstagep,
            tc.tile_pool(name="frwin", bufs=2) as frwinp,
            tc.tile_pool(name="zpre", bufs=2) as zprep,
            tc.tile_pool(name="state", bufs=1) as statep,
            tc.tile_pool(name="lwin", bufs=2) as lwinp,
        ):
            # ---- constants into SBUF
            wh_sb = constp.tile([128, NH * H], BF16)  # [hi_p, k*H + ho]
            nc.sync.dma_start(
                out=wh_sb[:].rearrange("p (k h) -> p k h", k=NH),
                in_=whT[:].rearrange("k p h -> p k h"),
            )
            wi_sb = constp.tile([I, H], BF16)
            nc.sync.dma_start(out=wi_sb[:], in_=wiT[:])
            wo_sb = constp.tile([128, NH * O], BF16)  # [h_p, k*O + o]
            nc.sync.dma_start(
                out=wo_sb[:].rearrange("p (k o) -> p k o", k=NH),
                in_=woT[:].rearrange("k p o -> p k o"),
            )
            bh_sb = constp.tile([128, NH], F32)
            nc.sync.dma_start(out=bh_sb[:], in_=bh[:])
            bo_sb = constp.tile([O, 1], F32)
            nc.sync.dma_start(out=bo_sb[:], in_=bo[:])
            zeros16 = constp.tile([128, BL], BF16)
            nc.vector.memset(zeros16[:], 0.0)

            # ---- phase 1: a*(u_proj.T)[h, (t,b)] + a*(b_in+b_h)
            # stored fp32 [p, c*NT + t*BL + b]
            uproj = uprojp.tile([128, NH * NT], F32)
            with tc.tile_pool(name="psum_u", bufs=2, space="PSUM") as psum_up:
                for n in range(NT // 512):
                    xs = xstagep.tile([I, 512], BF16)
                    nc.sync.dma_start(
                        out=xs[:], in_=xT[:, n * 512 : (n + 1) * 512]
                    )
                    for c in range(NH):
                        ps = psum_up.tile([128, 512], F32)
                        nc.tensor.matmul(
                            ps[:],
                            lhsT=wi_sb[:, c * 128 : (c + 1) * 128],
                            rhs=xs[:],
                            start=True,
                            stop=True,
                        )
                        nc.scalar.activation(
                            uproj[:, c * NT + n * 512 : c * NT + (n + 1) * 512],
                            ps[:],
                            mybir.ActivationFunctionType.Identity,
                            bias=bh_sb[:, c : c + 1],
                            scale=1.0,
                        )

            # ---- phase 2: recurrence
            v = statep.tile([128, NH * BL], F32)
            nc.vector.memset(v[:], 0.0)

            with (
                tc.tile_pool(name="psum_r", bufs=7, space="PSUM") as psum_rp,
                tc.tile_pool(name="psum_l", bufs=1, space="PSUM") as psum_lp,
            ):
                zp = None
                frw = None
                frw_prev = None
                for t in range(T):
                    w, tw = divmod(t, WIN)
                    if tw == 0:
                        frw_prev = frw
                        frw = frwinp.tile([128, NH * WIN * BL], BF16)

                    if t == 0:
                        # zp_0 = u_0 (v=0); computed as one strided op
                        zp = zprep.tile([128, NH * BL], F32)
                        nc.vector.scalar_tensor_tensor(
                            out=zp[:].rearrange("p (c b) -> p c b", c=NH),
                            in0=v[:].rearrange("p (c b) -> p c b", c=NH),
                            scalar=1.0 - ALPHA,
                            in1=uproj[:].rearrange("p (c n) -> p c n", c=NH)[
                                :, :, 0:BL
                            ],
                            op0=mult,
                            op1=add,
                        )
                    zp_cur = zp
                    zp_next = zprep.tile([128, NH * BL], F32)

                    for c in range(NH):
                        psr = psum_rp.tile([128, BL], F32, tag="psr")
                        for k in range(NH):
                            if t == 0:
                                rhs = zeros16[:]
                            elif tw == 0:
                                rhs = frw_prev[
                                    :,
                                    k * WIN * BL + (WIN - 1) * BL : k * WIN * BL
                                    + WIN * BL,
                                ]
                            else:
                                rhs = frw[
                                    :,
                                    k * WIN * BL + (tw - 1) * BL : k * WIN * BL
                                    + tw * BL,
                                ]
                            nc.tensor.matmul(
                                psr[:],
                                lhsT=wh_sb[
                                    :, k * H + c * 128 : k * H + (c + 1) * 128
                                ],
                                rhs=rhs,
                                start=(k == 0),
                                stop=(k == NH - 1),
                            )
                        # v_c = psum_c + zp_c
                        nc.vector.tensor_tensor(
                            out=v[:, c * BL : (c + 1) * BL],
                            in0=psr[:],
                            in1=zp_cur[:, c * BL : (c + 1) * BL],
                            op=add,
                        )
                        if c % 2 == 1:
                            # fr = relu(v) for chunks (c-1, c) -> bf16 window
                            # (one ScalarE op per chunk pair)
                            nc.scalar.activation(
                                frw[:].rearrange(
                                    "p (k s) -> p k s", k=NH
                                )[:, c - 1 : c + 1, tw * BL : (tw + 1) * BL],
                                v[:].rearrange("p (c b) -> p c b", c=NH)[
                                    :, c - 1 : c + 1, :
                                ],
                                mybir.ActivationFunctionType.Relu,
                            )
                    if t + 1 < T:
                        # zp_{t+1} = 0.9*v + u_{t+1}  (one batched DVE op,
                        # runs during the next step's matmuls)
                        nc.vector.scalar_tensor_tensor(
                            out=zp_next[:].rearrange("p (c b) -> p c b", c=NH),
                            in0=v[:].rearrange("p (c b) -> p c b", c=NH),
                            scalar=1.0 - ALPHA,
                            in1=uproj[:].rearrange("p (c n) -> p c n", c=NH)[
                                :, :, (t + 1) * BL : (t + 2) * BL
                            ],
                            op0=mult,
                            op1=add,
                        )
                    zp = zp_next

                    if tw == WIN - 1:
                        # logits window GEMM: [O, WIN*BL]
                        psl = psum_lp.tile([O, WIN * BL], F32)
                        for k in range(NH):
                            nc.tensor.matmul(
                                psl[:],
                                lhsT=wo_sb[:, k * O : (k + 1) * O],
                                rhs=frw[:, k * WIN * BL : (k + 1) * WIN * BL],
                                start=(k == 0),
                                stop=(k == NH - 1),
                            )
                        lw = lwinp.tile([O, WIN * BL], F32)
                        nc.scalar.activation(
                            lw[:],
                            psl[:],
                            mybir.ActivationFunctionType.Identity,
                            bias=bo_sb[:],
                            scale=1.0,
                        )
                        nc.sync.dma_start(out=logT[w], in_=lw[:])
                        nc.sync.dma_start(out=hseqT[w], in_=frw[:])

    _split_excess_waits(nc)
    return nc


_NC_CACHE = None


def _get_nc():
    global _NC_CACHE
    if _NC_CACHE is None:
        _NC_CACHE = _build_nc()
    return _NC_CACHE


def _make_in_maps(inputs) -> list[dict[str, np.ndarray]]:
    x = np.asarray(inputs["inputs"], np.float32)          # (T,B,I)
    W_in = np.asarray(inputs["W_in"], np.float32)         # (H,I)
    b_in = np.asarray(inputs["b_in"], np.float32)         # (H,)
    W_h = np.asarray(inputs["W_h"], np.float32)           # (H,H)
    b_h = np.asarray(inputs["b_h"], np.float32)           # (H,)
    W_out = np.asarray(inputs["W_out"], np.float32)       # (O,H)
    b_out = np.asarray(inputs["b_out"], np.float32)       # (O,)

    a = np.float32(ALPHA)
    whT = np.ascontiguousarray(
        (a * W_h).T.reshape(NH, 128, H)
    ).astype(NPBF)  # [hi_c, hi_p, ho]
    wiT = np.ascontiguousarray((a * W_in).T).astype(NPBF)  # (I,H)
    woT = np.ascontiguousarray(W_out.T.reshape(NH, 128, O)).astype(NPBF)
    bhv = (a * (b_in + b_h)).astype(np.float32)  # (H,)
    bh_arr = np.ascontiguousarray(bhv.reshape(NH, 128).T)  # [p, c]
    bo_arr = np.ascontiguousarray(b_out.reshape(O, 1))

    in_maps = []
    for core in range(NCORES):
        xc = x[:, core * BL : (core + 1) * BL, :]          # (T,BL,I)
        xTc = np.ascontiguousarray(xc.reshape(NT, I).T).astype(NPBF)
        in_maps.append(
            {
                "xT": xTc,
                "whT": whT,
                "wiT": wiT,
                "woT": woT,
                "bh": bh_arr,
                "bo": bo_arr,
            }
        )
    return in_maps


def kernel(**inputs) -> tuple[np.ndarray, np.ndarray]:
    in_maps = _make_in_maps(inputs)
    nc = _get_nc()
    res = run_bass_kernel_spmd(nc, in_maps, list(range(NCORES)))

    logits = np.empty((T, B, O), np.float32)
    hidden = np.empty((T, B, H), np.float32)
    for core in range(NCORES):
        r = res.results[core]
        # hseqT: [NW, 128, NH*WIN*BL] with free index (c, tw, b)
        hs = np.asarray(r["hseqT"], dtype=np.float32).reshape(
            NW, 128, NH, WIN, BL
        )  # [w, p, c, tw, b]
        hs = hs.transpose(0, 3, 4, 2, 1).reshape(T, BL, H)  # h = c*128 + p
        hidden[:, core * BL : (core + 1) * BL, :] = hs
        lg = np.asarray(r["logT"], dtype=np.float32).reshape(NW, O, WIN, BL)
        lg = lg.transpose(0, 2, 3, 1).reshape(T, BL, O)
        logits[:, core * BL : (core + 1) * BL, :] = lg

    return logits, hidden


# revision 12
# speedup vs baseline: 1.2602x; 1.2602x over previous
"""CTRNN kernel for 8 Trainium2 NeuronCores (Bass/Tile).

Model (per reference):
    u = inputs @ W_in.T + b_in                      # (T,B,H)
    v_t = (1-a) v_{t-1} + a (u_t + fr_{t-1} @ W_h.T + b_h)
    fr_t = relu(v_t)                                # hidden_seq
    logits = hidden_seq @ W_out.T + b_out           # (T,B,O)

Sharding: data-parallel over batch B=128 -> 16 per core; the T=512
recurrence runs sequentially per core. State is kept transposed on-chip
as [H on partitions (4 x 128 chunks), batch on free dim]. Matmul
operands are bf16 (fp32 PSUM accumulation, fp32 state update); fp32
matmuls on TRN2 lower to 2x half-rate passes and are ~11x slower.

Per-step schedule: 16 bf16 LDWEIGHTS+MATMUL pairs accumulate
a*W_h@fr.T into one PSUM bank; then exactly three batched DVE ops:
    v  = psum + zp          (tensor_tensor)
    fr = max(v, 0) -> bf16  (tensor_scalar_max, t-major window buffer)
    zp'= 0.9*v + a*u_{t+1}  (scalar_tensor_tensor, off critical path)
The input projection (u) and the logits GEMM are interleaved into the
loop so their big-N matmuls fill the PE gaps left by the serial chain.
"""

import sys

sys.path.insert(0, "/opt/trn_rl_repo")

import ml_dtypes
import numpy as np

import concourse.bass as bass
import concourse.mybir as mybir
from concourse.bass_utils import run_bass_kernel_spmd
from concourse.tile import TileContext

T, B, I, H, O = 512, 128, 64, 512, 32
ALPHA = 10.0 / 100.0
NCORES = 8
BL = B // NCORES          # 16 batch per core
NH = H // 128             # 4 h chunks of 128 partitions
WIN = 32                  # timesteps per window
NW = T // WIN             # 16 windows
NT = T * BL               # 8192 (t,b) rows per core
SW = WIN * BL             # 512 elements per window row
F32 = mybir.dt.float32
BF16 = mybir.dt.bfloat16
NPBF = ml_dtypes.bfloat16

# ---------------------------------------------------------------------------
# Workaround: this walrus build rejects instructions carrying more than one
# sem wait ("Too many sync wait commands"). After Tile scheduling, move
# excess waits onto same-engine NOPs inserted immediately before the
# offending instruction (same sequencer -> identical blocking semantics).
_MAX_WAITS = 1


def _split_excess_waits(nc: bass.Bass) -> None:
    count = 0
    for fn in nc.m.functions:
        for bb in fn.blocks:
            idx = 0
            while idx < len(bb.instructions):
                inst = bb.instructions[idx]
                si = inst.sync_info
                waits = list(si.on_wait) if si and si.on_wait else []
                if len(waits) > _MAX_WAITS:
                    keep = waits[-_MAX_WAITS:]
                    extra = waits[:-_MAX_WAITS]
                    inst.sync_info = mybir.SyncInfo(
                        on_wait=keep, on_update=list(si.on_update or [])
                    )
                    for w in extra:
                        count += 1
                        nop = mybir.InstNoOp(
                            name=f"I-waitsplit-{count}", ins=[], outs=[]
                        )
                        nop.engine = inst.engine
                        nop.sync_info = mybir.SyncInfo(on_wait=[w], on_update=[])
                        bb.instructions.insert(idx, nop)
                        idx += 1
                idx += 1
# ---------------------------------------------------------------------------

add = mybir.AluOpType.add
mult = mybir.AluOpType.mult


def _build_nc() -> bass.Bass:
    nc = bass.Bass()

    # Per-core inputs (already transposed/scaled/cast on host).
    xT = nc.declare_dram_parameter("xT", [I, NT], BF16, isOutput=False)
    whT = nc.declare_dram_parameter("whT", [NH, 128, H], BF16, isOutput=False)
    wiT = nc.declare_dram_parameter("wiT", [I, H], BF16, isOutput=False)
    woT = nc.declare_dram_parameter("woT", [NH, 128, O], BF16, isOutput=False)
    bh = nc.declare_dram_parameter("bh", [128, NH], F32, isOutput=False)
    bo = nc.declare_dram_parameter("bo", [O, 1], F32, isOutput=False)
    # hseqT free index is t-major: (tw, c, b)
    hseqT = nc.declare_dram_parameter(
        "hseqT", [NW, 128, WIN * NH * BL], BF16, isOutput=True
    )
    logT = nc.declare_dram_parameter(
        "logT", [NW, O, WIN * BL], F32, isOutput=True
    )

    with TileContext(nc) as tc:
        with (
            tc.tile_pool(name="const", bufs=1) as constp,
            tc.tile_pool(name="uproj", bufs=1) as uprojp,
            tc.tile_pool(name="xstage", bufs=2) as xstagep,
            tc.tile_pool(name="frwin", bufs=2) as frwinp,
            tc.tile_pool(name="zpre", bufs=2) as zprep,
            tc.tile_pool(name="state", bufs=1) as statep,
            tc.tile_pool(name="lwin", bufs=2) as lwinp,
            tc.tile_pool(name="psum_r", bufs=3, space="PSUM") as psum_rp,
            tc.tile_pool(name="psum_u", bufs=2, space="PSUM") as psum_up,
            tc.tile_pool(name="psum_l", bufs=1, space="PSUM") as psum_lp,
        ):
            # ---- constants into SBUF
            wh_sb = constp.tile([128, NH * H], BF16)  # [hi_p, k*H + ho]
            nc.sync.dma_start(
                out=wh_sb[:].rearrange("p (k h) -> p k h", k=NH),
                in_=whT[:].rearrange("k p h -> p k h"),
            )
            wi_sb = constp.tile([I, H], BF16)
            nc.sync.dma_start(out=wi_sb[:], in_=wiT[:])
            wo_sb = constp.tile([128, NH * O], BF16)  # [h_p, k*O + o]
            nc.sync.dma_start(
                out=wo_sb[:].rearrange("p (k o) -> p k o", k=NH),
                in_=woT[:].rearrange("k p o -> p k o"),
            )
            bh_sb = constp.tile([128, NH], F32)
            nc.sync.dma_start(out=bh_sb[:], in_=bh[:])
            bo_sb = constp.tile([O, 1], F32)
            nc.sync.dma_start(out=bo_sb[:], in_=bo[:])
            zeros16 = constp.tile([128, BL], BF16)
            nc.vector.memset(zeros16[:], 0.0)

            # u_proj buffer, t-major: [p, t*64 + c*16 + b], fp32
            uproj = uprojp.tile([128, NH * NT], F32)

            def u_tile(n, c):
                """One u_proj matmul + biased evacuation for n-tile n
                (timesteps [32n, 32n+32)), h-chunk c. xs must hold xT's
                columns [512n, 512(n+1))."""
                ps = psum_up.tile([128, 512], F32)
                nc.tensor.matmul(
                    ps[:],
                    lhsT=wi_sb[:, c * 128 : (c + 1) * 128],
                    rhs=xstage[n % 2][:],
                    start=True,
                    stop=True,
                )
                # psum free idx = (t_local, b); out at [p, t*64 + c*16 + b]
                nc.scalar.activation(
                    uproj[:].rearrange("p (s x) -> p s x", x=NH * BL)[
                        :, n * WIN : (n + 1) * WIN, c * BL : (c + 1) * BL
                    ],
                    ps[:].rearrange("p (s b) -> p s b", b=BL),
                    mybir.ActivationFunctionType.Identity,
                    bias=bh_sb[:, c : c + 1],
                    scale=1.0,
                )

            xstage = [None, None]

            def x_load(n):
                xstage[n % 2] = xstagep.tile([I, 512], BF16, name="xs", tag="xs")
                nc.sync.dma_start(
                    out=xstage[n % 2][:], in_=xT[:, n * 512 : (n + 1) * 512]
                )

            # bootstrap: u for t in [0, 64)
            for n in (0, 1):
                x_load(n)
                for c in range(NH):
                    u_tile(n, c)

            # ---- phase 2: recurrence
            v = statep.tile([128, NH * BL], F32)
            nc.vector.memset(v[:], 0.0)

            zp = None
            frw = None
            frw_prev = None
            for t in range(T):
                w, tw = divmod(t, WIN)
                if tw == 0:
                    frw_prev = frw
                    frw = frwinp.tile([128, WIN * NH * BL], BF16)
                    if w + 2 < NW:
                        x_load(w + 2)

                if t == 0:
                    # zp_0 = u_0 (v = 0)
                    zp = zprep.tile([128, NH * BL], F32)
                    nc.vector.scalar_tensor_tensor(
                        out=zp[:],
                        in0=v[:],
                        scalar=1.0 - ALPHA,
                        in1=uproj[:, 0 : NH * BL],
                        op0=mult,
                        op1=add,
                    )
                zp_cur = zp
                zp_next = zprep.tile([128, NH * BL], F32)

                # 16 accumulating matmuls into one PSUM bank
                psr = psum_rp.tile([128, 512], F32, tag="psr")
                for c in range(NH):
                    for k in range(NH):
                        if t == 0:
                            rhs = zeros16[:]
                        elif tw == 0:
                            rhs = frw_prev[
                                :,
                                (WIN - 1) * NH * BL + k * BL : (WIN - 1) * NH * BL
                                + (k + 1) * BL,
                            ]
                        else:
                            rhs = frw[
                                :,
                                (tw - 1) * NH * BL + k * BL : (tw - 1) * NH * BL
                                + (k + 1) * BL,
                            ]
                        nc.tensor.matmul(
                            psr[:, c * BL : (c + 1) * BL],
                            lhsT=wh_sb[
                                :, k * H + c * 128 : k * H + (c + 1) * 128
                            ],
                            rhs=rhs,
                            start=(k == 0),
                            stop=(k == NH - 1),
                        )

                # v = psum + zp ; fr = relu(v) (bf16, t-major window slice)
                nc.vector.tensor_tensor(
                    out=v[:], in0=psr[:, 0 : NH * BL], in1=zp_cur[:], op=add
                )
                nc.vector.tensor_scalar_max(
                    frw[:, tw * NH * BL : (tw + 1) * NH * BL], v[:], 0.0
                )
                if t + 1 < T:
                    nc.vector.scalar_tensor_tensor(
                        out=zp_next[:],
                        in0=v[:],
                        scalar=1.0 - ALPHA,
                        in1=uproj[:, (t + 1) * NH * BL : (t + 2) * NH * BL],
                        op0=mult,
                        op1=add,
                    )
                zp = zp_next

                # interleave next-next window's u_proj matmuls (big-N PE
                # work that fills the chain gap)
                if tw in (4, 12, 20, 28) and w + 2 < NW:
                    u_tile(w + 2, (tw - 4) // 8)

                if tw == WIN - 1:
                    # logits window GEMM: [O, WIN*BL]
                    frw3 = frw[:].rearrange("p (s x) -> p s x", x=NH * BL)
                    psl = psum_lp.tile([O, WIN * BL], F32)
                    for k in range(NH):
                        nc.tensor.matmul(
                            psl[:],
                            lhsT=wo_sb[:, k * O : (k + 1) * O],
                            rhs=frw3[:, :, k * BL : (k + 1) * BL],
                            start=(k == 0),
                            stop=(k == NH - 1),
                        )
                    lw = lwinp.tile([O, WIN * BL], F32)
                    nc.scalar.activation(
                        lw[:],
                        psl[:],
                        mybir.ActivationFunctionType.Identity,
                        bias=bo_sb[:],
                        scale=1.0,
                    )
                    nc.sync.dma_start(out=logT[w], in_=lw[:])
                    nc.sync.dma_start(out=hseqT[w], in_=frw[:])

    _split_excess_waits(nc)
    return nc


_NC_CACHE = None


def _get_nc():
    global _NC_CACHE
    if _NC_CACHE is None:
        _NC_CACHE = _build_nc()
    return _NC_CACHE


def _make_in_maps(inputs) -> list[dict[str, np.ndarray]]:
    x = np.asarray(inputs["inputs"], np.float32)          # (T,B,I)
    W_in = np.asarray(inputs["W_in"], np.float32)         # (H,I)
    b_in = np.asarray(inputs["b_in"], np.float32)         # (H,)
    W_h = np.asarray(inputs["W_h"], np.float32)           # (H,H)
    b_h = np.asarray(inputs["b_h"], np.float32)           # (H,)
    W_out = np.asarray(inputs["W_out"], np.float32)       # (O,H)
    b_out = np.asarray(inputs["b_out"], np.float32)       # (O,)

    a = np.float32(ALPHA)
    whT = np.ascontiguousarray(
        (a * W_h).T.reshape(NH, 128, H)
    ).astype(NPBF)  # [hi_c, hi_p, ho]
    wiT = np.ascontiguousarray((a * W_in).T).astype(NPBF)  # (I,H)
    woT = np.ascontiguousarray(W_out.T.reshape(NH, 128, O)).astype(NPBF)
    bhv = (a * (b_in + b_h)).astype(np.float32)  # (H,)
    bh_arr = np.ascontiguousarray(bhv.reshape(NH, 128).T)  # [p, c]
    bo_arr = np.ascontiguousarray(b_out.reshape(O, 1))

    in_maps = []
    for core in range(NCORES):
        xc = x[:, core * BL : (core + 1) * BL, :]          # (T,BL,I)
        xTc = np.ascontiguousarray(xc.reshape(NT, I).T).astype(NPBF)
        in_maps.append(
            {
                "xT": xTc,
                "whT": whT,
                "wiT": wiT,
                "woT": woT,
                "bh": bh_arr,
                "bo": bo_arr,
            }
        )
    return in_maps


def kernel(**inputs) -> tuple[np.ndarray, np.ndarray]:
    in_maps = _make_in_maps(inputs)
    nc = _get_nc()
    res = run_bass_kernel_spmd(nc, in_maps, list(range(NCORES)))

    logits = np.empty((T, B, O), np.float32)
    hidden = np.empty((T, B, H), np.float32)
    for core in range(NCORES):
        r = res.results[core]
        # hseqT: [NW, 128, WIN*NH*BL] with free index (tw, c, b)
        hs = np.asarray(r["hseqT"], dtype=np.float32).reshape(
            NW, 128, WIN, NH, BL
        )  # [w, p, tw, c, b]
        hs = hs.transpose(0, 2, 4, 3, 1).reshape(T, BL, H)  # h = c*128 + p
        hidden[:, core * BL : (core + 1) * BL, :] = hs
        lg = np.asarray(r["logT"], dtype=np.float32).reshape(NW, O, WIN, BL)
        lg = lg.transpose(0, 2, 3, 1).reshape(T, BL, O)
        logits[:, core * BL : (core + 1) * BL, :] = lg

    return logits, hidden


# revision 15
# speedup vs baseline: 1.2784x; 1.0145x over previous
"""CTRNN kernel for 8 Trainium2 NeuronCores (Bass/Tile).

Model (per reference):
    u = inputs @ W_in.T + b_in                      # (T,B,H)
    v_t = (1-a) v_{t-1} + a (u_t + fr_{t-1} @ W_h.T + b_h)
    fr_t = relu(v_t)                                # hidden_seq
    logits = hidden_seq @ W_out.T + b_out           # (T,B,O)

Sharding: data-parallel over batch B=128 -> 16 per core; the T=512
recurrence runs sequentially per core. State is kept transposed on-chip
as [H on partitions (4 x 128 chunks), batch on free dim]. Matmul
operands are bf16 (fp32 PSUM accumulation, fp32 state update); fp32
matmuls on TRN2 lower to 2x half-rate passes and are ~11x slower.

Per-step schedule: 16 bf16 LDWEIGHTS+MATMUL pairs accumulate
a*W_h@fr.T into one PSUM bank; then exactly three batched DVE ops:
    v  = psum + zp          (tensor_tensor)
    fr = max(v, 0) -> bf16  (tensor_scalar_max, t-major window buffer)
    zp'= 0.9*v + a*u_{t+1}  (scalar_tensor_tensor, off critical path)
The input projection (u) and the logits GEMM are interleaved into the
loop so their big-N matmuls fill the PE gaps left by the serial chain.
"""

import sys

sys.path.insert(0, "/opt/trn_rl_repo")

import ml_dtypes
import numpy as np

import concourse.bass as bass
import concourse.mybir as mybir
from concourse.bass_utils import run_bass_kernel_spmd
from concourse.tile import TileContext

T, B, I, H, O = 512, 128, 64, 512, 32
ALPHA = 10.0 / 100.0
NCORES = 8
BL = B // NCORES          # 16 batch per core
NH = H // 128             # 4 h chunks of 128 partitions
WIN = 32                  # timesteps per window
NW = T // WIN             # 16 windows
NT = T * BL               # 8192 (t,b) rows per core
SW = WIN * BL             # 512 elements per window row
F32 = mybir.dt.float32
BF16 = mybir.dt.bfloat16
NPBF = ml_dtypes.bfloat16

# ---------------------------------------------------------------------------
# Workaround: this walrus build rejects instructions carrying more than one
# sem wait ("Too many sync wait commands"). After Tile scheduling, move
# excess waits onto same-engine NOPs inserted immediately before the
# offending instruction (same sequencer -> identical blocking semantics).
_MAX_WAITS = 1


def _split_excess_waits(nc: bass.Bass) -> None:
    count = 0
    for fn in nc.m.functions:
        for bb in fn.blocks:
            idx = 0
            while idx < len(bb.instructions):
                inst = bb.instructions[idx]
                si = inst.sync_info
                waits = list(si.on_wait) if si and si.on_wait else []
                if len(waits) > _MAX_WAITS:
                    keep = waits[-_MAX_WAITS:]
                    extra = waits[:-_MAX_WAITS]
                    inst.sync_info = mybir.SyncInfo(
                        on_wait=keep, on_update=list(si.on_update or [])
                    )
                    for w in extra:
                        count += 1
                        nop = mybir.InstNoOp(
                            name=f"I-waitsplit-{count}", ins=[], outs=[]
                        )
                        nop.engine = inst.engine
                        nop.sync_info = mybir.SyncInfo(on_wait=[w], on_update=[])
                        bb.instructions.insert(idx, nop)
                        idx += 1
                idx += 1
# ---------------------------------------------------------------------------

add = mybir.AluOpType.add
mult = mybir.AluOpType.mult


def _build_nc() -> bass.Bass:
    nc = bass.Bass()

    # Per-core inputs (already transposed/scaled/cast on host).
    xT = nc.declare_dram_parameter("xT", [I, NT], BF16, isOutput=False)
    whT = nc.declare_dram_parameter("whT", [NH, 128, H], BF16, isOutput=False)
    wiT = nc.declare_dram_parameter("wiT", [I, H], BF16, isOutput=False)
    woT = nc.declare_dram_parameter("woT", [NH, 128, O], BF16, isOutput=False)
    bh = nc.declare_dram_parameter("bh", [128, NH], F32, isOutput=False)
    bo = nc.declare_dram_parameter("bo", [O, 1], F32, isOutput=False)
    # hseqT free index is t-major: (tw, c, b)
    hseqT = nc.declare_dram_parameter(
        "hseqT", [NW, 128, WIN * NH * BL], BF16, isOutput=True
    )
    logT = nc.declare_dram_parameter(
        "logT", [NW, O, WIN * BL], F32, isOutput=True
    )

    with TileContext(nc) as tc:
        with (
            tc.tile_pool(name="const", bufs=1) as constp,
            tc.tile_pool(name="uproj", bufs=1) as uprojp,
            tc.tile_pool(name="xstage", bufs=2) as xstagep,
            tc.tile_pool(name="frwin", bufs=2) as frwinp,
            tc.tile_pool(name="zpre", bufs=2) as zprep,
            tc.tile_pool(name="state", bufs=1) as statep,
            tc.tile_pool(name="lwin", bufs=2) as lwinp,
            tc.tile_pool(name="psum_r", bufs=3, space="PSUM") as psum_rp,
            tc.tile_pool(name="psum_u", bufs=2, space="PSUM") as psum_up,
            tc.tile_pool(name="psum_l", bufs=1, space="PSUM") as psum_lp,
        ):
            # ---- constants into SBUF
            wh_sb = constp.tile([128, NH * H], BF16)  # [hi_p, k*H + ho]
            nc.sync.dma_start(
                out=wh_sb[:].rearrange("p (k h) -> p k h", k=NH),
                in_=whT[:].rearrange("k p h -> p k h"),
            )
            wi_sb = constp.tile([I, H], BF16)
            nc.sync.dma_start(out=wi_sb[:], in_=wiT[:])
            wo_sb = constp.tile([128, NH * O], BF16)  # [h_p, k*O + o]
            nc.sync.dma_start(
                out=wo_sb[:].rearrange("p (k o) -> p k o", k=NH),
                in_=woT[:].rearrange("k p o -> p k o"),
            )
            bh_sb = constp.tile([128, NH], F32)
            nc.sync.dma_start(out=bh_sb[:], in_=bh[:])
            bo_sb = constp.tile([O, 1], F32)
            nc.sync.dma_start(out=bo_sb[:], in_=bo[:])
            zeros16 = constp.tile([128, BL], BF16)
            nc.vector.memset(zeros16[:], 0.0)

            # u_proj buffer, t-major: [p, t*64 + c*16 + b], fp32
            uproj = uprojp.tile([128, NH * NT], F32)

            def u_tile(n, c):
                """One u_proj matmul + biased evacuation for n-tile n
                (timesteps [32n, 32n+32)), h-chunk c. xs must hold xT's
                columns [512n, 512(n+1))."""
                ps = psum_up.tile([128, 512], F32)
                nc.tensor.matmul(
                    ps[:],
                    lhsT=wi_sb[:, c * 128 : (c + 1) * 128],
                    rhs=xstage[n % 2][:],
                    start=True,
                    stop=True,
                )
                # psum free idx = (t_local, b); out at [p, t*64 + c*16 + b]
                nc.scalar.activation(
                    uproj[:].rearrange("p (s x) -> p s x", x=NH * BL)[
                        :, n * WIN : (n + 1) * WIN, c * BL : (c + 1) * BL
                    ],
                    ps[:].rearrange("p (s b) -> p s b", b=BL),
                    mybir.ActivationFunctionType.Identity,
                    bias=bh_sb[:, c : c + 1],
                    scale=1.0,
                )

            xstage = [None, None]

            def x_load(n):
                xstage[n % 2] = xstagep.tile([I, 512], BF16, name="xs", tag="xs")
                nc.sync.dma_start(
                    out=xstage[n % 2][:], in_=xT[:, n * 512 : (n + 1) * 512]
                )

            # bootstrap: u for t in [0, 64)
            for n in (0, 1):
                x_load(n)
                for c in range(NH):
                    u_tile(n, c)

            # ---- phase 2: recurrence
            v = statep.tile([128, NH * BL], F32)
            nc.vector.memset(v[:], 0.0)

            zp = None
            frw = None
            frw_prev = None
            for t in range(T):
                w, tw = divmod(t, WIN)
                if tw == 0:
                    frw_prev = frw
                    frwA = frwinp.tile([128, WIN * NH * BL // 2], BF16)
                    frwB = frwinp.tile([128, WIN * NH * BL // 2], BF16)
                    frw = (frwA, frwB)
                    if w + 2 < NW:
                        x_load(w + 2)

                if t == 0:
                    # zp_0 = u_0 (v = 0)
                    zp = zprep.tile([128, NH * BL], F32)
                    nc.vector.scalar_tensor_tensor(
                        out=zp[:],
                        in0=v[:],
                        scalar=1.0 - ALPHA,
                        in1=uproj[:, 0 : NH * BL],
                        op0=mult,
                        op1=add,
                    )
                zp_cur = zp
                zp_next = zprep.tile([128, NH * BL], F32)

                # 16 accumulating matmuls into one PSUM bank
                psr = psum_rp.tile([128, 512], F32, tag="psr")
                for c in range(NH):
                    for k in range(NH):
                        if t == 0:
                            rhs = zeros16[:]
                        else:
                            src_w, src_tw = (frw_prev, WIN - 1) if tw == 0 else (frw, tw - 1)
                            buf = src_w[src_tw % 2]
                            off = (src_tw // 2) * NH * BL
                            rhs = buf[:, off + k * BL : off + (k + 1) * BL]
                        nc.tensor.matmul(
                            psr[:, c * BL : (c + 1) * BL],
                            lhsT=wh_sb[
                                :, k * H + c * 128 : k * H + (c + 1) * 128
                            ],
                            rhs=rhs,
                            start=(k == 0),
                            stop=(k == NH - 1),
                        )

                # v = psum + zp ; fr = relu(v) (bf16, t-major window slice)
                nc.vector.tensor_tensor(
                    out=v[:], in0=psr[:, 0 : NH * BL], in1=zp_cur[:], op=add
                )
                nc.vector.tensor_scalar_max(
                    frw[tw % 2][
                        :, (tw // 2) * NH * BL : (tw // 2 + 1) * NH * BL
                    ],
                    v[:],
                    0.0,
                )
                if t + 1 < T:
                    nc.vector.scalar_tensor_tensor(
                        out=zp_next[:],
                        in0=v[:],
                        scalar=1.0 - ALPHA,
                        in1=uproj[:, (t + 1) * NH * BL : (t + 2) * NH * BL],
                        op0=mult,
                        op1=add,
                    )
                zp = zp_next

                # interleave next-next window's u_proj matmuls (big-N PE
                # work that fills the chain gap)
                if tw in (4, 12, 20, 28) and w + 2 < NW:
                    u_tile(w + 2, (tw - 4) // 8)

                if tw == WIN - 1:
                    # logits window GEMM: [O, WIN*BL] over both parity bufs
                    psl = psum_lp.tile([O, WIN * BL], F32)
                    for par in range(2):
                        f3 = frw[par][:].rearrange(
                            "p (s x) -> p s x", x=NH * BL
                        )
                        for k in range(NH):
                            nc.tensor.matmul(
                                psl[:].rearrange("o (s q) -> o s q", q=2 * BL)[
                                    :, :, par * BL : (par + 1) * BL
                                ],
                                lhsT=wo_sb[:, k * O : (k + 1) * O],
                                rhs=f3[:, :, k * BL : (k + 1) * BL],
                                start=(k == 0),
                                stop=(k == NH - 1),
                            )
                    lw = lwinp.tile([O, WIN * BL], F32)
                    nc.scalar.activation(
                        lw[:],
                        psl[:],
                        mybir.ActivationFunctionType.Identity,
                        bias=bo_sb[:],
                        scale=1.0,
                    )
                    nc.sync.dma_start(out=logT[w], in_=lw[:])
                    nc.sync.dma_start(out=hseqT[w, :, 0 : WIN * NH * BL // 2], in_=frwA[:])
                    nc.sync.dma_start(
                        out=hseqT[w, :, WIN * NH * BL // 2 : WIN * NH * BL],
                        in_=frwB[:],
                    )

    _split_excess_waits(nc)
    return nc


_NC_CACHE = None


def _get_nc():
    global _NC_CACHE
    if _NC_CACHE is None:
        _NC_CACHE = _build_nc()
    return _NC_CACHE


def _make_in_maps(inputs) -> list[dict[str, np.ndarray]]:
    x = np.asarray(inputs["inputs"], np.float32)          # (T,B,I)
    W_in = np.asarray(inputs["W_in"], np.float32)         # (H,I)
    b_in = np.asarray(inputs["b_in"], np.float32)         # (H,)
    W_h = np.asarray(inputs["W_h"], np.float32)           # (H,H)
    b_h = np.asarray(inputs["b_h"], np.float32)           # (H,)
    W_out = np.asarray(inputs["W_out"], np.float32)       # (O,H)
    b_out = np.asarray(inputs["b_out"], np.float32)       # (O,)

    a = np.float32(ALPHA)
    whT = np.ascontiguousarray(
        (a * W_h).T.reshape(NH, 128, H)
    ).astype(NPBF)  # [hi_c, hi_p, ho]
    wiT = np.ascontiguousarray((a * W_in).T).astype(NPBF)  # (I,H)
    woT = np.ascontiguousarray(W_out.T.reshape(NH, 128, O)).astype(NPBF)
    bhv = (a * (b_in + b_h)).astype(np.float32)  # (H,)
    bh_arr = np.ascontiguousarray(bhv.reshape(NH, 128).T)  # [p, c]
    bo_arr = np.ascontiguousarray(b_out.reshape(O, 1))

    in_maps = []
    for core in range(NCORES):
        xc = x[:, core * BL : (core + 1) * BL, :]          # (T,BL,I)
        xTc = np.ascontiguousarray(xc.reshape(NT, I).T).astype(NPBF)
        in_maps.append(
            {
                "xT": xTc,
                "whT": whT,
                "wiT": wiT,
                "woT": woT,
                "bh": bh_arr,
                "bo": bo_arr,
            }
        )
    return in_maps


def kernel(**inputs) -> tuple[np.ndarray, np.ndarray]:
    in_maps = _make_in_maps(inputs)
    nc = _get_nc()
    res = run_bass_kernel_spmd(nc, in_maps, list(range(NCORES)))

    logits = np.empty((T, B, O), np.float32)
    hidden = np.empty((T, B, H), np.float32)
    for core in range(NCORES):
        r = res.results[core]
        # hseqT: [NW, 128, WIN*NH*BL]; first half = even tw (parity buffer
        # A), second half = odd tw; within each: (tw//2, c, b)
        hs = np.asarray(r["hseqT"], dtype=np.float32).reshape(
            NW, 128, 2, WIN // 2, NH, BL
        )  # [w, p, par, s, c, b] with tw = 2*s + par
        hs = hs.transpose(0, 3, 2, 5, 4, 1).reshape(T, BL, H)  # h = c*128+p
        hidden[:, core * BL : (core + 1) * BL, :] = hs
        lg = np.asarray(r["logT"], dtype=np.float32).reshape(NW, O, WIN, BL)
        lg = lg.transpose(0, 2, 3, 1).reshape(T, BL, O)
        logits[:, core * BL : (core + 1) * BL, :] = lg

    return logits, hidden
